# revision 27
# baseline (speedup 1.0000x reference)
"""Causal self-attention (B=4, T=2048, C=1024, H=16) on 8 Trainium2 cores.

Sharding: core c = (batch b = c//2, head-group g = c%2 covering 8 heads).
Each core computes QKV for its 8 heads, causal flash attention, and a
partial output projection (its 512 rows of w_proj). Host sums the two
partial projections per batch element and adds b_proj.

Per-core kernel (Bass/Tile on Bacc):
  - All matmul operands are bf16 (inputs pre-cast host-side): halves HBM
    traffic and SBUF footprint at ~5e-3 final l2 error (gate is 2e-2).
  - QKV chunks (512 tokens each) produce kT/qT (feature-major) and v
    (token-major, with a ones column for softmax sums); q and its bias
    pre-scaled by 1/sqrt(dh) host-side.  x chunks double-buffered; w_qk
    streamed in per-feature-block chunks so the first matmul starts after
    ~4us.
  - Attention row-blocks I (512 queries) interleave with QKV chunks:
    block I only needs chunks <= I, so attention (ScalarE-heavy exp)
    overlaps QKV/projection matmuls (PE-heavy).  Scores are computed
    transposed (s^T = K @ Q^T, [key, query] layout); the two heads of a
    pair use disjoint PE row-groups (partitions 0-63 / 64-127) and write
    the two halves of one 2-bank PSUM tile, so softmax needs ONE exp
    instruction per key-tile.  No max-subtraction (|s| = O(8) here).
    The causal mask is a [128,128] triangular additive tile applied to
    diagonal key-tiles on the (otherwise idle) Pool engine; below-diagonal
    query columns are never computed (sliced matmuls/exp/PV).
  - y^T and the softmax denominators come out of one PV matmul per key
    tile (ones column -> PSUM row 64); 1/l is broadcast across partitions
    by Pool's partition_broadcast and multiplied in on VectorE.
  - Projection: out = y^T.T @ w_proj_shard, DMA per 128 rows.
"""

import os
from contextlib import ExitStack

import numpy as np
import ml_dtypes

import concourse.bass as bass
import concourse.bacc as bacc
import concourse.tile as tile
from concourse import mybir
from concourse.bass_utils import run_bass_kernel_spmd

B, T, C = 4, 2048, 1024
H, DH = 16, 64
NCORES = 8
HLOC = 8  # heads per core
P = 128
NEG = -1.0e30

f32 = mybir.dt.float32
bf16 = mybir.dt.bfloat16
BF_NP = ml_dtypes.bfloat16

ts = bass.ts

_PROGRAM = None
LAST_RESULTS = None


def _emit(ctx: ExitStack, tc: tile.TileContext, ins: dict, out: bass.AP):
    nc = tc.nc
    NT = T // P          # 16 token tiles
    NCH = T // 512       # 4 token chunks == 4 query row-blocks

    xT_d = ins["xT"].rearrange("(co ci) t -> ci co t", ci=P)        # [128, 8, 2048]
    wqk_d = ins["w_qk"].rearrange("(co ci) f -> ci co f", ci=P)     # [128, 8, 1024]
    wv_d = ins["w_v"].rearrange("(co ci) f -> ci co f", ci=P)       # [128, 8, 512]
    wproj_d = ins["w_proj"].rearrange("(co ci) f -> ci co f", ci=P) # [128, 4, 1024]

    singles = ctx.enter_context(tc.tile_pool(name="singles", bufs=1))
    kT = singles.tile([P, 4, T], bf16)            # [p, hp, t]
    v_sb = singles.tile([P, NT, HLOC, DH + 1], bf16)
    yT = singles.tile([P, 4, T], bf16)            # [p, kp, t] local head feats
    bqk_sb = singles.tile([P, 8], f32)
    bv_sb = singles.tile([P, HLOC, DH], f32)
    tri_sb = singles.tile([P, P], f32)            # tri[k,q]=0 if k<=q else -1e30

    nc.vector.memset(v_sb[:], 1.0)  # col DH stays 1.0 -> softmax sums

    ps_mm = ctx.enter_context(tc.tile_pool(name="ps_mm", bufs=2, space="PSUM"))
    ps_s = ctx.enter_context(tc.tile_pool(name="ps_s", bufs=2, space="PSUM"))
    ps_yv = ctx.enter_context(tc.tile_pool(name="ps_yv", bufs=2, space="PSUM"))
    pt_pool = ctx.enter_context(tc.tile_pool(name="pt_pool", bufs=4))
    small = ctx.enter_context(tc.tile_pool(name="small", bufs=4))

    qtiles = [None] * NCH

    def qkv_units(wqk_sb, wv_sb, x_pool, q_pool, ch, split_dma=False):
        state = {}

        def prelude():
            x_t = x_pool.tile([P, 8, 512], bf16, tag="x")
            nc.sync.dma_start(x_t[:], xT_d[:, :, ts(ch, 512)])
            q_t = q_pool.tile([P, 4, 512], bf16)
            state["x"] = x_t
            qtiles[ch] = q_t

        def ft_unit(ft):
            def u():
                x_t = state["x"]
                ps = ps_mm.tile([P, 512], f32, tag="mm")
                for c in range(8):
                    nc.tensor.matmul(
                        ps[:],
                        lhsT=wqk_sb[:, c, ts(ft, P)],
                        rhs=x_t[:, c, :],
                        start=(c == 0),
                        stop=(c == 7),
                    )
                dst = (
                    qtiles[ch][:, ft, :]
                    if ft < 4
                    else kT[:, ft - 4, ts(ch, 512)]
                )
                nc.vector.tensor_tensor(
                    dst,
                    ps[:],
                    bqk_sb[:, ft : ft + 1].to_broadcast([P, 512]),
                    mybir.AluOpType.add,
                )
            return u

        def v_unit(sub):
            def u():
                x_t = state["x"]
                tt = ch * 4 + sub
                ps = ps_mm.tile([P, 512], f32, tag="mm")
                for c in range(8):
                    nc.tensor.matmul(
                        ps[:],
                        lhsT=x_t[:, c, ts(sub, P)],
                        rhs=wv_sb[:, c, :],
                        start=(c == 0),
                        stop=(c == 7),
                    )
                nc.vector.tensor_tensor(
                    v_sb[:, tt, :, :DH],
                    ps[:].rearrange("p (h d) -> p h d", h=HLOC),
                    bv_sb[:],
                    mybir.AluOpType.add,
                )
            return u

        return (
            [prelude]
            + [ft_unit(ft) for ft in range(8)]
            + [v_unit(sub) for sub in range(4)]
        )

    def attn_units(I, final=False):
        """Fine-grained units: one per (head-pair, key-tile j) plus one
        epilogue per pair, so dense PE filler work can be interleaved at j
        granularity (the exp chain makes ScalarE the per-j pacing engine)."""
        njs = 4 * (I + 1)
        units = []
        for hp in range(4):
            state = {}

            def j_unit(hp=hp, j=0, state=state):
                def u():
                    if j == 0:
                        state["yvs"] = [
                            ps_yv.tile([DH + 1, 512], f32, tag="yv", name=f"yv{s}")
                            for s in range(2)
                        ]
                    q_t = qtiles[I]
                    yvs = state["yvs"]
                    r = j - 4 * I  # >=0: diagonal key-tile
                    q0 = 128 * r if r > 0 else 0
                    # The two heads of the pair use disjoint PE row-groups
                    # (partitions 0-63 / 64-127) and the two halves of one
                    # 2-bank PSUM tile, so one exp covers both.
                    sp = ps_s.tile([P, 2, 512], f32, tag="sp", name="sp")
                    for sub in range(2):
                        po = 64 * sub
                        nc.tensor.matmul(
                            sp[:, sub, q0:],
                            lhsT=kT[po : po + 64, hp, ts(j, P)],
                            rhs=q_t[po : po + 64, hp, q0:],
                            start=True,
                            stop=True,
                        )
                    if r >= 0:
                        # DVE, not Pool: GPSIMD cannot access PSUM.
                        nc.vector.tensor_tensor(
                            sp[:, :, q0 : q0 + P],
                            sp[:, :, q0 : q0 + P],
                            tri_sb[:].rearrange("p (o q) -> p o q", o=1)
                            .to_broadcast([P, 2, P]),
                            mybir.AluOpType.add,
                        )
                    pt = pt_pool.tile([P, 2, 512], bf16, tag="pt", name="pt")
                    nc.scalar.activation(
                        pt[:, :, q0:], sp[:, :, q0:],
                        mybir.ActivationFunctionType.Exp,
                    )
                    for sub in range(2):
                        h = 2 * hp + sub
                        nc.tensor.matmul(
                            yvs[sub][:, q0:],
                            lhsT=v_sb[:, j, h, :],
                            rhs=pt[:, sub, q0:],
                            start=(j == 0),
                            stop=(j == njs - 1),
                        )
                return u

            def epilogue(hp=hp, state=state):
                def u():
                    yvs = state["yvs"]
                    direct = final and hp == 3
                    if direct:
                        # Last pair of the last block: nobody needs the PSUM
                        # banks next, so skip the eviction copies and run the
                        # (latency-critical) chain straight off PSUM.
                        ysbs = yvs
                    else:
                        # Copies first: frees both PSUM banks for the next
                        # pair before the slow recip/broadcast/mult chain.
                        ysbs = []
                        for sub in range(2):
                            ysb = small.tile([DH + 1, 512], f32, tag="ysb")
                            nc.vector.tensor_copy(ysb[:], yvs[sub][:])
                            ysbs.append(ysb)
                    for sub in range(2):
                        po = 64 * sub
                        ysb = ysbs[sub]
                        linv = small.tile([1, 512], f32)
                        nc.vector.reciprocal(linv[:], ysb[DH : DH + 1, :])
                        linb = small.tile([64, 512], f32, tag="linb")
                        nc.gpsimd.partition_broadcast(linb[:], linv[:])
                        nc.vector.tensor_tensor(
                            yT[po : po + 64, hp, ts(I, 512)],
                            ysb[:DH, :],
                            linb[:],
                            mybir.AluOpType.mult,
                        )
                return u

            units += [j_unit(hp, j, state) for j in range(njs)]
            units.append(epilogue(hp, state))
        return units

    def proj_units(wproj_sb, out_pool):
        """Two half-units per token tile (4 matmuls + evict each) so proj
        can interleave at j granularity with attention."""
        states = [{} for _ in range(NT)]

        def half_unit(tt, n):
            def u():
                st = states[tt]
                if n == 0:
                    st["o"] = out_pool.tile([P, 1024], bf16, tag="o", name="o")
                o_t = st["o"]
                ps = ps_mm.tile([P, 512], f32, tag="mm")
                for kp in range(4):
                    nc.tensor.matmul(
                        ps[:],
                        lhsT=yT[:, kp, ts(tt, P)],
                        rhs=wproj_sb[:, kp, ts(n, 512)],
                        start=(kp == 0),
                        stop=(kp == 3),
                    )
                nc.vector.tensor_copy(o_t[:, ts(n, 512)], ps[:])
                if n == 1:
                    nc.sync.dma_start(out[ts(tt, P), :], o_t[:])
            return u

        return [half_unit(tt, n) for tt in range(NT) for n in range(2)]

    def proj_tail_units(wproj_sb, out_pool):
        """Token tiles 12-15 depend on the very last attention epilogue via
        their kp=3 slice only, so accumulate kp 0-2 first (ready earlier) and
        finish with kp=3 + evict once the epilogue lands."""
        states = {}

        def a_unit(tt, n):
            def u():
                if n == 0:
                    states[tt] = {
                        "o": out_pool.tile([P, 1024], bf16, tag="o", name="o")
                    }
                ps = ps_mm.tile([P, 512], f32, tag="mm")
                states[tt][n] = ps
                for kp in range(3):
                    nc.tensor.matmul(
                        ps[:],
                        lhsT=yT[:, kp, ts(tt, P)],
                        rhs=wproj_sb[:, kp, ts(n, 512)],
                        start=(kp == 0),
                        stop=False,
                    )
            return u

        def b_unit(tt, n):
            def u():
                st = states[tt]
                ps = st[n]
                nc.tensor.matmul(
                    ps[:],
                    lhsT=yT[:, 3, ts(tt, P)],
                    rhs=wproj_sb[:, 3, ts(n, 512)],
                    start=False,
                    stop=True,
                )
                nc.vector.tensor_copy(st["o"][:, ts(n, 512)], ps[:])
                if n == 1:
                    nc.sync.dma_start(out[ts(tt, P), :], st["o"][:])
            return u

        units = []
        for tt in range(12, NT):
            units += [a_unit(tt, 0), a_unit(tt, 1), b_unit(tt, 0), b_unit(tt, 1)]
        return units

    def interleave(a, b):
        """Merge unit lists proportionally (emission order ~ priority)."""
        out = []
        na, nb = len(a), len(b)
        ia = ib = 0
        while ia < na or ib < nb:
            if (ib * na <= ia * nb and ib < nb) or ia >= na:
                out.append(b[ib]); ib += 1
            else:
                out.append(a[ia]); ia += 1
        return out

    def run(units):
        for u in units:
            u()

    with tc.tile_pool(name="q_pool", bufs=3) as q_pool:
        with (
            tc.tile_pool(name="wqk_pool", bufs=1) as wqk_pool,
            tc.tile_pool(name="x_pool", bufs=2) as x_pool,
        ):
            wqk_sb = wqk_pool.tile([P, 8, 1024], bf16)
            wv_sb = wqk_pool.tile([P, 8, 512], bf16)
            # DMA order = first-needed bytes first (DMA engines serialize at
            # HBM bandwidth): tiny qk-bias, x chunk 0, w_qk per-feature-block
            # chunks (ft_unit(0) starts after x + 0.25 MB), then w_v + the
            # rest of the small tensors.
            nc.sync.dma_start(bqk_sb[:], ins["b_qk"][:])
            ch0 = qkv_units(wqk_sb, wv_sb, x_pool, q_pool, 0)
            ch0[0]()  # x chunk 0 DMA
            for ft in range(8):  # first-needed feature block arrives first
                nc.sync.dma_start(wqk_sb[:, :, ts(ft, P)], wqk_d[:, :, ts(ft, P)])
            nc.sync.dma_start(wv_sb[:], wv_d[:])
            nc.sync.dma_start(bv_sb[:], ins["b_v"][:])
            nc.sync.dma_start(tri_sb[:], ins["tri"][:])
            run(ch0[1:])  # ft-units already precede v-units

            run(interleave(attn_units(0),
                           qkv_units(wqk_sb, wv_sb, x_pool, q_pool, 1)))
            run(interleave(attn_units(1),
                           qkv_units(wqk_sb, wv_sb, x_pool, q_pool, 2)))
            run(interleave(attn_units(2),
                           qkv_units(wqk_sb, wv_sb, x_pool, q_pool, 3)))

        with tc.tile_pool(name="proj_pool", bufs=1) as proj_pool, tc.tile_pool(
            name="out_pool", bufs=3
        ) as out_pool:
            wproj_sb = proj_pool.tile([P, 4, 1024], bf16)
            nc.sync.dma_start(wproj_sb[:], wproj_d[:])

            pu = proj_units(wproj_sb, out_pool)
            # tt 0-11 (24 half-units) interleave into attention block 3 as
            # PE filler; tt 12-15 need block 3's yT and run as the tail.
            run(interleave(attn_units(3, final=True), pu[:24]))
            run(pu[24:])


def _build_program():
    global _PROGRAM
    if _PROGRAM is not None:
        return _PROGRAM
    nc = bacc.Bacc(
        "TRN2", target_bir_lowering=False, debug=False, num_devices=NCORES
    )
    ins = {
        "xT": nc.dram_tensor("xT", [C, T], bf16, kind="ExternalInput").ap(),
        "w_qk": nc.dram_tensor("w_qk", [C, 1024], bf16, kind="ExternalInput").ap(),
        "w_v": nc.dram_tensor("w_v", [C, 512], bf16, kind="ExternalInput").ap(),
        "w_proj": nc.dram_tensor("w_proj", [512, C], bf16, kind="ExternalInput").ap(),
        "b_qk": nc.dram_tensor("b_qk", [P, 8], f32, kind="ExternalInput").ap(),
        "b_v": nc.dram_tensor("b_v", [P, HLOC, DH], f32, kind="ExternalInput").ap(),
        "tri": nc.dram_tensor("tri", [P, P], f32, kind="ExternalInput").ap(),
    }
    out = nc.dram_tensor("out", [T, C], bf16, kind="ExternalOutput").ap()
    with tile.TileContext(nc) as tc:
        with ExitStack() as ctx:
            _emit(ctx, tc, ins, out)
    nc.compile()
    _PROGRAM = nc
    return nc


def _make_in_maps(x, w_qkv, b_qkv, w_proj):
    scale = 1.0 / np.sqrt(DH)
    kk = np.arange(P)[:, None]
    qq = np.arange(P)[None, :]
    tri = np.where(kk <= qq, 0.0, NEG).astype(np.float32)

    in_maps = []
    for core in range(NCORES):
        b, g = divmod(core, 2)
        lo, hi = g * 512, (g + 1) * 512
        w_q = w_qkv[:, lo:hi] * scale
        w_k = w_qkv[:, C + lo : C + hi]
        w_v = w_qkv[:, 2 * C + lo : 2 * C + hi]
        b_q = b_qkv[lo:hi] * scale
        b_k = b_qkv[C + lo : C + hi]
        b_v = b_qkv[2 * C + lo : 2 * C + hi]
        in_maps.append(
            {
                "xT": np.ascontiguousarray(x[b].T.astype(BF_NP)),
                "w_qk": np.ascontiguousarray(
                    np.concatenate([w_q, w_k], axis=1).astype(BF_NP)
                ),
                "w_v": np.ascontiguousarray(w_v.astype(BF_NP)),
                "w_proj": np.ascontiguousarray(w_proj[lo:hi, :].astype(BF_NP)),
                "b_qk": np.ascontiguousarray(
                    np.concatenate([b_q, b_k]).reshape(8, P).T, dtype=np.float32
                ),
                "b_v": np.ascontiguousarray(
                    np.broadcast_to(b_v.reshape(1, HLOC, DH), (P, HLOC, DH)),
                    dtype=np.float32,
                ),
                "tri": tri,
            }
        )
    return in_maps


def kernel(x, w_qkv, b_qkv, w_proj, b_proj):
    global LAST_RESULTS
    x = np.asarray(x, dtype=np.float32)
    w_qkv = np.asarray(w_qkv, dtype=np.float32)
    b_qkv = np.asarray(b_qkv, dtype=np.float32)
    w_proj = np.asarray(w_proj, dtype=np.float32)
    b_proj = np.asarray(b_proj, dtype=np.float32)

    nc = _build_program()
    in_maps = _make_in_maps(x, w_qkv, b_qkv, w_proj)
    res = run_bass_kernel_spmd(
        nc,
        in_maps,
        list(range(NCORES)),
        trace=bool(int(os.environ.get("KERNEL_TRACE", "0"))),
    )
    LAST_RESULTS = res

    out = np.empty((B, T, C), dtype=np.float32)
    for b in range(B):
        out[b] = (
            res.results[2 * b]["out"].astype(np.float32)
            + res.results[2 * b + 1]["out"].astype(np.float32)
            + b_proj
        )
    return out


# revision 34
# speedup vs baseline: 267.7439x; 267.7439x over previous
"""Causal self-attention (B=4, T=2048, C=1024, H=16) on 8 Trainium2 cores.

Sharding: core c = (batch b = c//2, head-group g = c%2 covering 8 heads).
Each core computes QKV for its 8 heads, causal flash attention, and a
partial output projection (its 512 rows of w_proj). Host sums the two
partial projections per batch element and adds b_proj.

Per-core kernel (Bass/Tile on Bacc):
  - All matmul operands are bf16 (inputs pre-cast host-side): halves HBM
    traffic and SBUF footprint at ~5e-3 final l2 error (gate is 2e-2).
  - QKV chunks (512 tokens each) produce kT/qT (feature-major) and v
    (token-major, with a ones column for softmax sums); q and its bias
    pre-scaled by 1/sqrt(dh) host-side.  x chunks double-buffered; w_qk
    streamed in per-feature-block chunks so the first matmul starts after
    ~4us.
  - Attention row-blocks I (512 queries) interleave with QKV chunks:
    block I only needs chunks <= I, so attention (ScalarE-heavy exp)
    overlaps QKV/projection matmuls (PE-heavy).  Scores are computed
    transposed (s^T = K @ Q^T, [key, query] layout); the two heads of a
    pair use disjoint PE row-groups (partitions 0-63 / 64-127) and write
    the two halves of one 2-bank PSUM tile, so softmax needs ONE exp
    instruction per key-tile.  No max-subtraction (|s| = O(8) here).
    The causal mask is a [128,128] triangular additive tile applied to
    diagonal key-tiles on the (otherwise idle) Pool engine; below-diagonal
    query columns are never computed (sliced matmuls/exp/PV).
  - y^T and the softmax denominators come out of one PV matmul per key
    tile (ones column -> PSUM row 64); 1/l is broadcast across partitions
    by Pool's partition_broadcast and multiplied in on VectorE.
  - Projection: out = y^T.T @ w_proj_shard, DMA per 128 rows.
"""

import os
from contextlib import ExitStack

import numpy as np
import ml_dtypes

import concourse.bass as bass
import concourse.bacc as bacc
import concourse.tile as tile
from concourse import mybir
from concourse.bass_utils import run_bass_kernel_spmd

B, T, C = 4, 2048, 1024
H, DH = 16, 64
NCORES = 8
HLOC = 8  # heads per core
P = 128
NEG = -1.0e30

f32 = mybir.dt.float32
bf16 = mybir.dt.bfloat16
BF_NP = ml_dtypes.bfloat16

ts = bass.ts

_PROGRAM = None
LAST_RESULTS = None


def _emit(ctx: ExitStack, tc: tile.TileContext, ins: dict, out: bass.AP):
    nc = tc.nc
    NT = T // P          # 16 token tiles
    NCH = T // 512       # 4 token chunks == 4 query row-blocks

    xT_d = ins["xT"].rearrange("(co ci) t -> ci co t", ci=P)        # [128, 8, 2048]
    wqk_d = ins["w_qk"].rearrange("(co ci) f -> ci co f", ci=P)     # [128, 8, 1024]
    wv_d = ins["w_v"].rearrange("(co ci) f -> ci co f", ci=P)       # [128, 8, 512]
    wproj_d = ins["w_proj"].rearrange("(co ci) f -> ci co f", ci=P) # [128, 4, 1024]

    singles = ctx.enter_context(tc.tile_pool(name="singles", bufs=1))
    kT = singles.tile([P, 4, T], bf16)            # [p, hp, t]
    v_sb = singles.tile([P, NT, HLOC, DH + 1], bf16)
    yT = singles.tile([P, 4, T], bf16)            # [p, kp, t] local head feats
    bqk_sb = singles.tile([P, 8], f32)
    bv_sb = singles.tile([P, HLOC, DH], f32)
    tri_sb = singles.tile([P, P], f32)            # tri[k,q]=0 if k<=q else -1e30

    nc.vector.memset(v_sb[:], 1.0)  # col DH stays 1.0 -> softmax sums

    ps_mm = ctx.enter_context(tc.tile_pool(name="ps_mm", bufs=2, space="PSUM"))
    ps_s = ctx.enter_context(tc.tile_pool(name="ps_s", bufs=2, space="PSUM"))
    ps_yv = ctx.enter_context(tc.tile_pool(name="ps_yv", bufs=2, space="PSUM"))
    pt_pool = ctx.enter_context(tc.tile_pool(name="pt_pool", bufs=4))
    small = ctx.enter_context(tc.tile_pool(name="small", bufs=4))

    qtiles = [None] * NCH

    def qkv_units(wqk_sb, wv_sb, x_pool, q_pool, ch, split_dma=False):
        state = {}

        def prelude():
            x_t = x_pool.tile([P, 8, 512], bf16, tag="x")
            if split_dma:
                # DMA order = first-needed bytes first: half of x, the first
                # w_qk feature block (unblocks ft_unit(0)'s c=0..3 matmuls),
                # then the rest of x.
                nc.sync.dma_start(x_t[:, :4, :], xT_d[:, :4, ts(ch, 512)])
                nc.sync.dma_start(
                    wqk_sb[:, :4, ts(0, P)], wqk_d[:, :4, ts(0, P)]
                )
                nc.sync.dma_start(x_t[:, 4:, :], xT_d[:, 4:, ts(ch, 512)])
                nc.sync.dma_start(
                    wqk_sb[:, 4:, ts(0, P)], wqk_d[:, 4:, ts(0, P)]
                )
            else:
                nc.sync.dma_start(x_t[:], xT_d[:, :, ts(ch, 512)])
            q_t = q_pool.tile([P, 4, 512], bf16)
            state["x"] = x_t
            qtiles[ch] = q_t

        def ft_unit(ft):
            def u():
                x_t = state["x"]
                ps = ps_mm.tile([P, 512], f32, tag="mm")
                for c in range(8):
                    nc.tensor.matmul(
                        ps[:],
                        lhsT=wqk_sb[:, c, ts(ft, P)],
                        rhs=x_t[:, c, :],
                        start=(c == 0),
                        stop=(c == 7),
                    )
                dst = (
                    qtiles[ch][:, ft, :]
                    if ft < 4
                    else kT[:, ft - 4, ts(ch, 512)]
                )
                nc.vector.tensor_tensor(
                    dst,
                    ps[:],
                    bqk_sb[:, ft : ft + 1].to_broadcast([P, 512]),
                    mybir.AluOpType.add,
                )
            return u

        def v_unit(sub):
            def u():
                x_t = state["x"]
                tt = ch * 4 + sub
                ps = ps_mm.tile([P, 512], f32, tag="mm")
                for c in range(8):
                    nc.tensor.matmul(
                        ps[:],
                        lhsT=x_t[:, c, ts(sub, P)],
                        rhs=wv_sb[:, c, :],
                        start=(c == 0),
                        stop=(c == 7),
                    )
                nc.vector.tensor_tensor(
                    v_sb[:, tt, :, :DH],
                    ps[:].rearrange("p (h d) -> p h d", h=HLOC),
                    bv_sb[:],
                    mybir.AluOpType.add,
                )
            return u

        return (
            [prelude]
            + [ft_unit(ft) for ft in range(8)]
            + [v_unit(sub) for sub in range(4)]
        )

    def attn_units(I, final=False):
        """Fine-grained units: one per (head-pair, key-tile j) plus one
        epilogue per pair, so dense PE filler work can be interleaved at j
        granularity (the exp chain makes ScalarE the per-j pacing engine)."""
        njs = 4 * (I + 1)
        units = []
        for hp in range(4):
            state = {}

            def j_unit(hp=hp, j=0, state=state):
                def u():
                    if j == 0:
                        state["yvs"] = [
                            ps_yv.tile([DH + 1, 512], f32, tag="yv", name=f"yv{s}")
                            for s in range(2)
                        ]
                    q_t = qtiles[I]
                    yvs = state["yvs"]
                    r = j - 4 * I  # >=0: diagonal key-tile
                    q0 = 128 * r if r > 0 else 0
                    # The two heads of the pair use disjoint PE row-groups
                    # (partitions 0-63 / 64-127) and the two halves of one
                    # 2-bank PSUM tile, so one exp covers both.
                    sp = ps_s.tile([P, 2, 512], f32, tag="sp", name="sp")
                    for sub in range(2):
                        po = 64 * sub
                        nc.tensor.matmul(
                            sp[:, sub, q0:],
                            lhsT=kT[po : po + 64, hp, ts(j, P)],
                            rhs=q_t[po : po + 64, hp, q0:],
                            start=True,
                            stop=True,
                        )
                    if r >= 0:
                        # DVE, not Pool: GPSIMD cannot access PSUM.
                        nc.vector.tensor_tensor(
                            sp[:, :, q0 : q0 + P],
                            sp[:, :, q0 : q0 + P],
                            tri_sb[:].rearrange("p (o q) -> p o q", o=1)
                            .to_broadcast([P, 2, P]),
                            mybir.AluOpType.add,
                        )
                    pt = pt_pool.tile([P, 2, 512], bf16, tag="pt", name="pt")
                    nc.scalar.activation(
                        pt[:, :, q0:], sp[:, :, q0:],
                        mybir.ActivationFunctionType.Exp,
                    )
                    for sub in range(2):
                        h = 2 * hp + sub
                        nc.tensor.matmul(
                            yvs[sub][:, q0:],
                            lhsT=v_sb[:, j, h, :],
                            rhs=pt[:, sub, q0:],
                            start=(j == 0),
                            stop=(j == njs - 1),
                        )
                return u

            def epilogue(hp=hp, state=state):
                def u():
                    yvs = state["yvs"]
                    direct = final and hp == 3
                    if direct:
                        # Last pair of the last block: nobody needs the PSUM
                        # banks next, so skip the eviction copies and run the
                        # (latency-critical) chain straight off PSUM.
                        ysbs = yvs
                    else:
                        # Copies first: frees both PSUM banks for the next
                        # pair before the slow recip/broadcast/mult chain.
                        ysbs = []
                        for sub in range(2):
                            ysb = small.tile([DH + 1, 512], f32, tag="ysb")
                            nc.vector.tensor_copy(ysb[:], yvs[sub][:])
                            ysbs.append(ysb)
                    for sub in range(2):
                        po = 64 * sub
                        ysb = ysbs[sub]
                        linv = small.tile([1, 512], f32)
                        nc.vector.reciprocal(linv[:], ysb[DH : DH + 1, :])
                        linb = small.tile([64, 512], f32, tag="linb")
                        nc.gpsimd.partition_broadcast(linb[:], linv[:])
                        nc.vector.tensor_tensor(
                            yT[po : po + 64, hp, ts(I, 512)],
                            ysb[:DH, :],
                            linb[:],
                            mybir.AluOpType.mult,
                        )
                return u

            units += [j_unit(hp, j, state) for j in range(njs)]
            units.append(epilogue(hp, state))
        return units

    def proj_units(wproj_sb, out_pool):
        """Two half-units per token tile (4 matmuls + evict each) so proj
        can interleave at j granularity with attention."""
        states = [{} for _ in range(NT)]

        def half_unit(tt, n):
            def u():
                st = states[tt]
                if n == 0:
                    st["o"] = out_pool.tile([P, 1024], bf16, tag="o", name="o")
                o_t = st["o"]
                ps = ps_mm.tile([P, 512], f32, tag="mm")
                for kp in range(4):
                    nc.tensor.matmul(
                        ps[:],
                        lhsT=yT[:, kp, ts(tt, P)],
                        rhs=wproj_sb[:, kp, ts(n, 512)],
                        start=(kp == 0),
                        stop=(kp == 3),
                    )
                nc.vector.tensor_copy(o_t[:, ts(n, 512)], ps[:])
                if n == 1:
                    nc.sync.dma_start(out[ts(tt, P), :], o_t[:])
            return u

        return [half_unit(tt, n) for tt in range(NT) for n in range(2)]

    def proj_tail_units(wproj_sb, out_pool):
        """Token tiles 12-15 depend on the very last attention epilogue via
        their kp=3 slice only, so accumulate kp 0-2 first (ready earlier) and
        finish with kp=3 + evict once the epilogue lands."""
        states = {}

        def a_unit(tt, n):
            def u():
                if n == 0:
                    states[tt] = {
                        "o": out_pool.tile([P, 1024], bf16, tag="o", name="o")
                    }
                ps = ps_mm.tile([P, 512], f32, tag="mm")
                states[tt][n] = ps
                for kp in range(3):
                    nc.tensor.matmul(
                        ps[:],
                        lhsT=yT[:, kp, ts(tt, P)],
                        rhs=wproj_sb[:, kp, ts(n, 512)],
                        start=(kp == 0),
                        stop=False,
                    )
            return u

        def b_unit(tt, n):
            def u():
                st = states[tt]
                ps = st[n]
                nc.tensor.matmul(
                    ps[:],
                    lhsT=yT[:, 3, ts(tt, P)],
                    rhs=wproj_sb[:, 3, ts(n, 512)],
                    start=False,
                    stop=True,
                )
                nc.vector.tensor_copy(st["o"][:, ts(n, 512)], ps[:])
                if n == 1:
                    nc.sync.dma_start(out[ts(tt, P), :], st["o"][:])
            return u

        units = []
        for tt in range(12, NT):
            units += [a_unit(tt, 0), a_unit(tt, 1), b_unit(tt, 0), b_unit(tt, 1)]
        return units

    def interleave(a, b):
        """Merge unit lists proportionally (emission order ~ priority)."""
        out = []
        na, nb = len(a), len(b)
        ia = ib = 0
        while ia < na or ib < nb:
            if (ib * na <= ia * nb and ib < nb) or ia >= na:
                out.append(b[ib]); ib += 1
            else:
                out.append(a[ia]); ia += 1
        return out

    def run(units):
        for u in units:
            u()

    with (
        tc.tile_pool(name="q_pool", bufs=3) as q_pool,
        tc.tile_pool(name="wqk_pool", bufs=1) as wqk_pool,
        tc.tile_pool(name="x_pool", bufs=2) as x_pool,
        tc.tile_pool(name="proj_pool", bufs=1) as proj_pool,
        tc.tile_pool(name="out_pool", bufs=3) as out_pool,
    ):
        wqk_sb = wqk_pool.tile([P, 8, 1024], bf16)
        wv_sb = wqk_pool.tile([P, 8, 512], bf16)
        wproj_sb = proj_pool.tile([P, 4, 1024], bf16)
        # DMA order = first-needed bytes first (DMA engines serialize at
        # HBM bandwidth): tiny qk-bias, x chunk 0, w_qk per-feature-block
        # chunks (ft_unit(0) starts after x + 0.25 MB), then w_v + the
        # rest of the small tensors.
        nc.sync.dma_start(bqk_sb[:], ins["b_qk"][:])
        # PE warm-up: dummy matmuls on a memset tile keep the PE busy
        # through the initial DMA wait so the clock-gate ramp (and the
        # cost model's p-state) is at full speed for the first real
        # matmuls.  Results are never read.
        warm = singles.tile([P, 64], bf16)
        nc.gpsimd.memset(warm[:], 1.0)
        for i in range(56):
            wps = ps_mm.tile([64, 64], f32, tag="mm")
            nc.tensor.matmul(
                wps[:], lhsT=warm[:], rhs=warm[:], start=True, stop=True
            )
        ch0 = qkv_units(wqk_sb, wv_sb, x_pool, q_pool, 0, split_dma=True)
        ch0[0]()  # x chunk 0 + w_qk ft-block 0, first-needed first
        for ft in range(1, 8):
            nc.sync.dma_start(wqk_sb[:, :, ts(ft, P)], wqk_d[:, :, ts(ft, P)])
        nc.sync.dma_start(wv_sb[:], wv_d[:])
        nc.sync.dma_start(bv_sb[:], ins["b_v"][:])
        nc.sync.dma_start(tri_sb[:], ins["tri"][:])
        run(ch0[1:])  # ft-units already precede v-units

        def wproj_dma():
            nc.sync.dma_start(wproj_sb[:], wproj_d[:])

        run(interleave(attn_units(0),
                       qkv_units(wqk_sb, wv_sb, x_pool, q_pool, 1)))
        run(interleave(attn_units(1),
                       qkv_units(wqk_sb, wv_sb, x_pool, q_pool, 2)))
        ch3 = qkv_units(wqk_sb, wv_sb, x_pool, q_pool, 3)
        # w_proj streams in behind chunk 3's x so it is resident long
        # before the first projection matmul.
        run(interleave(attn_units(2), ch3[:1] + [wproj_dma] + ch3[1:]))

        pu = proj_units(wproj_sb, out_pool)
        # tt 0-11 (24 half-units) interleave into attention block 3 as
        # PE filler; tt 12-15 need block 3's yT and run as the tail.
        run(interleave(attn_units(3, final=True), pu[:24]))
        run(pu[24:])


def _build_program():
    global _PROGRAM
    if _PROGRAM is not None:
        return _PROGRAM
    nc = bacc.Bacc(
        "TRN2", target_bir_lowering=False, debug=False, num_devices=NCORES
    )
    ins = {
        "xT": nc.dram_tensor("xT", [C, T], bf16, kind="ExternalInput").ap(),
        "w_qk": nc.dram_tensor("w_qk", [C, 1024], bf16, kind="ExternalInput").ap(),
        "w_v": nc.dram_tensor("w_v", [C, 512], bf16, kind="ExternalInput").ap(),
        "w_proj": nc.dram_tensor("w_proj", [512, C], bf16, kind="ExternalInput").ap(),
        "b_qk": nc.dram_tensor("b_qk", [P, 8], f32, kind="ExternalInput").ap(),
        "b_v": nc.dram_tensor("b_v", [P, HLOC, DH], f32, kind="ExternalInput").ap(),
        "tri": nc.dram_tensor("tri", [P, P], f32, kind="ExternalInput").ap(),
    }
    out = nc.dram_tensor("out", [T, C], bf16, kind="ExternalOutput").ap()
    with tile.TileContext(nc) as tc:
        with ExitStack() as ctx:
            _emit(ctx, tc, ins, out)
    nc.compile()
    _PROGRAM = nc
    return nc


def _make_in_maps(x, w_qkv, b_qkv, w_proj):
    scale = 1.0 / np.sqrt(DH)
    kk = np.arange(P)[:, None]
    qq = np.arange(P)[None, :]
    tri = np.where(kk <= qq, 0.0, NEG).astype(np.float32)

    in_maps = []
    for core in range(NCORES):
        b, g = divmod(core, 2)
        lo, hi = g * 512, (g + 1) * 512
        w_q = w_qkv[:, lo:hi] * scale
        w_k = w_qkv[:, C + lo : C + hi]
        w_v = w_qkv[:, 2 * C + lo : 2 * C + hi]
        b_q = b_qkv[lo:hi] * scale
        b_k = b_qkv[C + lo : C + hi]
        b_v = b_qkv[2 * C + lo : 2 * C + hi]
        in_maps.append(
            {
                "xT": np.ascontiguousarray(x[b].T.astype(BF_NP)),
                "w_qk": np.ascontiguousarray(
                    np.concatenate([w_q, w_k], axis=1).astype(BF_NP)
                ),
                "w_v": np.ascontiguousarray(w_v.astype(BF_NP)),
                "w_proj": np.ascontiguousarray(w_proj[lo:hi, :].astype(BF_NP)),
                "b_qk": np.ascontiguousarray(
                    np.concatenate([b_q, b_k]).reshape(8, P).T, dtype=np.float32
                ),
                "b_v": np.ascontiguousarray(
                    np.broadcast_to(b_v.reshape(1, HLOC, DH), (P, HLOC, DH)),
                    dtype=np.float32,
                ),
                "tri": tri,
            }
        )
    return in_maps


def kernel(x, w_qkv, b_qkv, w_proj, b_proj):
    global LAST_RESULTS
    x = np.asarray(x, dtype=np.float32)
    w_qkv = np.asarray(w_qkv, dtype=np.float32)
    b_qkv = np.asarray(b_qkv, dtype=np.float32)
    w_proj = np.asarray(w_proj, dtype=np.float32)
    b_proj = np.asarray(b_proj, dtype=np.float32)

    nc = _build_program()
    in_maps = _make_in_maps(x, w_qkv, b_qkv, w_proj)
    res = run_bass_kernel_spmd(
        nc,
        in_maps,
        list(range(NCORES)),
        trace=bool(int(os.environ.get("KERNEL_TRACE", "0"))),
    )
    LAST_RESULTS = res

    out = np.empty((B, T, C), dtype=np.float32)
    for b in range(B):
        out[b] = (
            res.results[2 * b]["out"].astype(np.float32)
            + res.results[2 * b + 1]["out"].astype(np.float32)
            + b_proj
        )
    return out
